# revision 16
# baseline (speedup 1.0000x reference)
"""Trainium2 Bass kernel for CRF loss (MLP emissions + CRF log-likelihood).

Sharding: data-parallel over B=256 sentences -> 32 per core on 8 cores.
Each core computes llh[32] (per-sentence log-likelihood); host sums and
scales (the "all-reduce" of the scalar loss is the trivial host gather).

CRF denominator: forward algorithm rewritten as a product of 3x3
per-step matrices in linear (exp) domain, reduced with a binary tree
(9 levels for T=512) with per-level max-rescaling (scales accumulate in
log domain) for numerical stability.
"""

import sys

sys.path.insert(0, "/opt/trn_rl_repo")

import numpy as np
from contextlib import ExitStack

import concourse.bass as bass
import concourse.mybir as mybir
import concourse.tile as tile
from concourse.masks import make_identity
from concourse import bass_utils

F32 = mybir.dt.float32
BF16 = mybir.dt.bfloat16
I32 = mybir.dt.int32
AF = mybir.ActivationFunctionType
OP = mybir.AluOpType
AX = mybir.AxisListType

BS, T, D, H, K = 32, 512, 512, 256, 3  # per-core shard
NCORES = 8


def build(trans, start, end, b2, mlp_only=False):
    trans = np.asarray(trans, np.float64)
    start = np.asarray(start, np.float64)
    end = np.asarray(end, np.float64)
    b2 = np.asarray(b2, np.float64)

    nc = bass.Bass()
    xt_d = nc.dram_tensor("xt", [D, BS, T], F32, kind="ExternalInput")
    tg_d = nc.dram_tensor("tags", [BS, T], I32, kind="ExternalInput")
    ln_d = nc.dram_tensor("lengths", [BS], I32, kind="ExternalInput")
    w1_d = nc.dram_tensor("W1", [D, H], F32, kind="ExternalInput")
    b1_d = nc.dram_tensor("b1", [H], F32, kind="ExternalInput")
    w2_d = nc.dram_tensor("W2", [H, K], F32, kind="ExternalInput")
    out_d = nc.dram_tensor("out", [BS], F32, kind="ExternalOutput")
    em_dram = nc.dram_tensor("em_scratch", [K, BS, T], BF16, kind="Internal")

    with tile.TileContext(nc) as tc, ExitStack() as ctx:
        consts = ctx.enter_context(tc.tile_pool(name="consts", bufs=1))
        xt_p = ctx.enter_context(tc.tile_pool(name="xt", bufs=2))
        g_p = ctx.enter_context(tc.tile_pool(name="g", bufs=2))
        ps_h = ctx.enter_context(tc.tile_pool(name="ps_h", bufs=2, space="PSUM"))
        ps_e = ctx.enter_context(tc.tile_pool(name="ps_e", bufs=2, space="PSUM"))
        tree_p = ctx.enter_context(tc.tile_pool(name="tree", bufs=2))
        m0_p = ctx.enter_context(tc.tile_pool(name="m0", bufs=1))
        sm_p = ctx.enter_context(tc.tile_pool(name="small", bufs=2))

        # ---- constants / weights ----
        w1f = consts.tile([128, 4, H], F32)
        nc.sync.dma_start(w1f[:], w1_d[:].rearrange("(dc p) h -> p dc h", p=128))
        w1b = consts.tile([128, 4, H], BF16)
        nc.vector.tensor_copy(w1b[:], w1f[:])
        w2f = consts.tile([128, 2, K], F32)
        nc.sync.dma_start(w2f[:], w2_d[:].rearrange("(hc p) k -> p hc k", p=128))
        w2b = consts.tile([128, 2, K], BF16)
        nc.vector.tensor_copy(w2b[:], w2f[:])
        b1sb = consts.tile([128, 2], F32)
        nc.sync.dma_start(b1sb[:], b1_d[:].rearrange("(hc p) -> p hc", p=128))
        pre_b1 = consts.tile([128, 2], F32)
        nc.scalar.copy(pre_b1[:], b1sb[:])

        em_sb = consts.tile([K, BS * T], BF16)

        # ---- MLP: per sentence (512 tokens) ----
        xt_r = xt_d[:].rearrange("(dc p) b t -> p dc b t", p=128)
        for b in range(BS):
            xT = xt_p.tile([128, 4, T], BF16)  # [d128, dc, tok]
            nc.gpsimd.dma_start(xT[:], xt_r[:, :, b, :])
            g = g_p.tile([128, 2, T], BF16)
            for ht in range(2):
                ph = ps_h.tile([128, T], F32)
                for dc in range(4):
                    nc.tensor.matmul(
                        ph[:], lhsT=w1b[:, dc, ht * 128:(ht + 1) * 128],
                        rhs=xT[:, dc, :], start=(dc == 0), stop=(dc == 3))
                nc.scalar.activation(g[:, ht, :], ph[:], AF.Gelu,
                                     bias=b1sb[:, ht:ht + 1])
            pe = ps_e.tile([K, T], F32)
            for ht in range(2):
                nc.tensor.matmul(pe[:], lhsT=w2b[:, ht, :], rhs=g[:, ht, :],
                                 start=(ht == 0), stop=(ht == 1))
            nc.scalar.copy(em_sb[:, b * T:(b + 1) * T], pe[:])

        # bounce em through DRAM to get [b, k, t] layout (partition = sentence)
        nc.sync.dma_start(em_dram[:].rearrange("k b t -> k (b t)"), em_sb[:])
        em_p = consts.tile([BS, K, T], BF16)
        nc.sync.dma_start(em_p[:], em_dram[:].rearrange("k b t -> b k t"))
        pre_em = consts.tile([BS, 1], BF16)
        nc.scalar.copy(pre_em[:], em_p[:, 0, 0:1])

        if mlp_only:
            zz = consts.tile([BS, 1], F32)
            nc.vector.tensor_copy(zz[:], em_p[:, 0, 0:1])
            nc.sync.dma_start(out_d[:].rearrange("(b o) -> b o", o=1), zz[:])
            return nc
        # ---- masks / tags ----
        im_i = consts.tile([BS, T], I32)
        nc.gpsimd.iota(im_i[:], pattern=[[1, T]], base=0, channel_multiplier=0)
        imf = consts.tile([BS, T], F32)
        nc.vector.tensor_copy(imf[:], im_i[:])
        li = consts.tile([BS, 1], I32)
        nc.sync.dma_start(li[:], ln_d[:].rearrange("(b o) -> b o", o=1))
        lf0 = consts.tile([BS, 1], F32)
        nc.vector.tensor_copy(lf0[:], li[:])
        lf = consts.tile([BS, 1], F32)
        nc.vector.tensor_scalar_max(lf[:], lf0[:], 1.0)
        m1 = consts.tile([BS, T], F32)
        nc.vector.tensor_scalar(m1[:], imf[:], lf[:, 0:1], None, OP.is_lt)
        m2 = consts.tile([BS, T], F32)
        nc.vector.tensor_scalar(m2[:], imf[:], 0.0, None, OP.is_gt)
        mp = consts.tile([BS, T], F32)
        nc.vector.tensor_mul(mp[:], m1[:], m2[:])
        omm = consts.tile([BS, T], F32)  # 1 - mp
        nc.vector.tensor_scalar(omm[:], mp[:], -1.0, 1.0, OP.mult, OP.add)

        tg_i = consts.tile([BS, T], I32)
        nc.sync.dma_start(tg_i[:], tg_d[:])
        tgf = consts.tile([BS, T], F32)
        nc.vector.tensor_copy(tgf[:], tg_i[:])

        # ---- CRF denominator on 128 partitions: (sentence, quarter) ----
        em_p128 = consts.tile([128, K, 128], BF16)
        nc.sync.dma_start(em_p128[:],
                          em_dram[:].rearrange("k b (q t) -> (b q) k t", q=4))
        pre_em2 = consts.tile([128, 1], BF16)
        nc.scalar.copy(pre_em2[:], em_p128[:, 0, 0:1])
        im_dram = nc.dram_tensor("im_scratch", [BS, T], F32, kind="Internal")
        nc.sync.dma_start(im_dram[:], imf[:])
        imf128 = consts.tile([128, 128], F32)
        nc.sync.dma_start(imf128[:],
                          im_dram[:].rearrange("b (q t) -> (b q) t", q=4))
        m1_dram = nc.dram_tensor("m1_scratch", [BS, T], F32, kind="Internal")
        nc.sync.dma_start(m1_dram[:], m1[:])
        m1b = consts.tile([128, 128], F32)
        nc.sync.dma_start(m1b[:],
                          m1_dram[:].rearrange("b (q t) -> (b q) t", q=4))
        m2b = consts.tile([128, 128], F32)
        nc.vector.tensor_scalar(m2b[:], imf128[:], 0.0, None, OP.is_gt)
        mpb = consts.tile([128, 128], F32)
        nc.vector.tensor_mul(mpb[:], m1b[:], m2b[:])
        ommb = consts.tile([128, 128], F32)
        nc.vector.tensor_scalar(ommb[:], mpb[:], -1.0, 1.0, OP.mult, OP.add)

        M0 = m0_p.tile([128, 128, 9], F32, tag="lvl0")
        trb = consts.tile([128, 9], F32)
        for i in range(K):
            for j in range(K):
                nc.vector.memset(trb[:, 3 * i + j:3 * i + j + 1],
                                 float(trans[i, j] + b2[j]))
        for i in range(K):
            for j in range(K):
                nc.scalar.activation(M0[:, :, 3 * i + j], em_p128[:, j, :],
                                     AF.Exp, bias=trb[:, 3 * i + j:3 * i + j + 1])
        for c in range(9):
            nc.vector.tensor_mul(M0[:, :, c], M0[:, :, c], mpb[:])
        for c in (0, 4, 8):
            nc.vector.tensor_add(M0[:, :, c], M0[:, :, c], ommb[:])

        def tree(cur, curN, P, ls, ls_pairs, rescale_at):
            while curN > 1:
                N = curN // 2
                nxt = tree_p.tile([P, max(N, 1), 9], F32, tag=f"nxt{P}")
                tmp = tree_p.tile([P, max(N, 1), 3], F32, tag=f"tmp{P}")
                cur_r = cur[:, 0:curN, :].rearrange(
                    "p (q two) e -> p q two e", two=2)
                B_r = cur_r[:, :, 1, :].rearrange(
                    "p q (kk jj) -> p q jj kk", jj=3)
                for i in range(3):
                    for j in range(3):
                        nc.vector.tensor_mul(
                            tmp[:], cur_r[:, :, 0, 3 * i:3 * i + 3],
                            B_r[:, :, j, :])
                        nc.vector.tensor_reduce(
                            nxt[:, :, 3 * i + j], tmp[:], axis=AX.X, op=OP.add)
                if N in rescale_at:
                    mx = sm_p.tile([P, N], F32, tag=f"mx{P}")
                    nc.vector.reduce_max(mx[:], nxt[:], axis=AX.X)
                    rc = sm_p.tile([P, N], F32, tag=f"rc{P}")
                    nc.vector.reciprocal(rc[:], mx[:])
                    for c in range(9):
                        nc.vector.tensor_mul(nxt[:, :, c], nxt[:, :, c], rc[:])
                    lg = sm_p.tile([P, N], F32, tag=f"lg{P}")
                    nc.scalar.activation(lg[:], mx[:], AF.Ln)
                    if ls is None:
                        ls = lg
                        ls_pairs = N
                    else:
                        ls_n = sm_p.tile([P, N], F32, tag=f"lsn{P}")
                        fold = ls_pairs // N
                        ls_r = ls[:, 0:ls_pairs].rearrange(
                            "p (q k) -> p q k", k=fold)
                        nc.vector.tensor_add(ls_n[:], ls_r[:, :, 0],
                                             ls_r[:, :, 1])
                        for kk in range(2, fold):
                            nc.vector.tensor_add(ls_n[:], ls_n[:],
                                                 ls_r[:, :, kk])
                        nc.vector.tensor_add(ls_n[:], ls_n[:], lg[:])
                        ls = ls_n
                        ls_pairs = N
                cur, curN = nxt, N
            return cur, ls

        cur128, ls128 = tree(M0, 128, 128, None, 0, {64, 16, 4, 1})
        fold_dram = nc.dram_tensor("fold_scratch", [128, 10], F32,
                                  kind="Internal")
        nc.sync.dma_start(fold_dram[:, 0:9], cur128[:, 0, :])
        nc.sync.dma_start(fold_dram[:, 9:10], ls128[:])
        G4 = consts.tile([BS, 4, 9], F32)
        nc.sync.dma_start(
            G4[:], fold_dram[:].rearrange("(b q) m -> b q m", q=4)[:, :, 0:9])
        ls4 = consts.tile([BS, 4], F32)
        nc.sync.dma_start(
            ls4[:], fold_dram[:].rearrange("(b q) m -> b q m", q=4)[:, :, 9])
        ls32 = sm_p.tile([BS, 1], F32, tag="ls32")
        nc.vector.tensor_reduce(ls32[:], ls4[:], axis=AX.X, op=OP.add)
        cur, lsf = tree(G4, 4, BS, None, 0, {1})
        ls = sm_p.tile([BS, 1], F32, tag="lsfin")
        nc.vector.tensor_add(ls[:], ls32[:], lsf[:, 0:1])

        # ---- numerator ----
        ind3 = consts.tile([BS, T, 3], F32)
        for j in range(3):
            nc.vector.tensor_scalar(ind3[:, :, j], tgf[:], float(j), None,
                                    OP.is_equal)
        tmpn = consts.tile([BS, T, 3], F32)
        nc.vector.tensor_mul(tmpn[:], em_p[:].rearrange("p j t -> p t j"),
                             ind3[:])
        emtag = consts.tile([BS, T], F32)
        nc.vector.tensor_reduce(emtag[:], tmpn[:], axis=AX.X, op=OP.add)
        if np.any(b2 != 0):
            b2s = consts.tile([BS, T], F32)
            nc.vector.tensor_scalar(b2s[:], ind3[:, :, 0], float(b2[0]), None,
                                    OP.mult)
            for j in (1, 2):
                u = sm_p.tile([BS, T], F32, tag="scr")
                nc.vector.tensor_scalar(u[:], ind3[:, :, j], float(b2[j]), None,
                                        OP.mult)
                nc.vector.tensor_add(b2s[:], b2s[:], u[:])
            nc.vector.tensor_add(emtag[:], emtag[:], b2s[:])

        prevf = consts.tile([BS, T], F32)
        nc.vector.memset(prevf[:, 0:1], 0.0)
        nc.vector.tensor_copy(prevf[:, 1:T], tgf[:, 0:T - 1])
        idxf = consts.tile([BS, T], F32)
        nc.vector.scalar_tensor_tensor(idxf[:], prevf[:], 3.0, tgf[:],
                                       OP.mult, OP.add)
        tr = consts.tile([BS, T], F32)
        tf = trans.reshape(9)
        nc.vector.tensor_scalar(tr[:], idxf[:], 0.0, float(tf[0]),
                                OP.is_equal, OP.mult)
        for p in range(1, 9):
            u2 = sm_p.tile([BS, T], F32, tag="scr")
            nc.vector.tensor_scalar(u2[:], idxf[:], float(p), float(tf[p]),
                                    OP.is_equal, OP.mult)
            nc.vector.tensor_add(tr[:], tr[:], u2[:])
        nc.vector.tensor_add(tr[:], tr[:], emtag[:])
        scrap = consts.tile([BS, T], F32)
        numsum = sm_p.tile([BS, 1], F32, tag="numsum")
        nc.vector.tensor_mul(scrap[:], tr[:], mp[:])
        nc.vector.tensor_reduce(numsum[:], scrap[:], axis=AX.X, op=OP.add)

        startc = consts.tile([BS, 3], F32)
        for i in range(3):
            nc.vector.memset(startc[:, i:i + 1], float(start[i]))
        scr3 = sm_p.tile([BS, 3], F32, tag="scr3")
        firstv = sm_p.tile([BS, 1], F32, tag="firstv")
        nc.vector.tensor_mul(scr3[:], ind3[:, 0, :], startc[:])
        nc.vector.tensor_reduce(firstv[:], scr3[:], axis=AX.X, op=OP.add)

        endv = consts.tile([BS, T], F32)
        nc.vector.tensor_scalar(endv[:], tgf[:], 0.0, float(end[0]),
                                OP.is_equal, OP.mult)
        for j in (1, 2):
            u3 = sm_p.tile([BS, T], F32, tag="scr")
            nc.vector.tensor_scalar(u3[:], tgf[:], float(j), float(end[j]),
                                    OP.is_equal, OP.mult)
            nc.vector.tensor_add(endv[:], endv[:], u3[:])
        indL = consts.tile([BS, T], F32)
        nc.vector.tensor_scalar(indL[:], imf[:], lf[:, 0:1], -1.0,
                                OP.subtract, OP.is_equal)
        lastv = sm_p.tile([BS, 1], F32, tag="lastv")
        nc.vector.tensor_mul(scrap[:], endv[:], indL[:])
        nc.vector.tensor_reduce(lastv[:], scrap[:], axis=AX.X, op=OP.add)

        # ---- final: alpha0 through G, combine ----
        s0c = consts.tile([BS, 3], F32)
        for i in range(3):
            nc.vector.memset(s0c[:, i:i + 1], float(start[i] + b2[i]))
        s0 = sm_p.tile([BS, 3], F32, tag="s0")
        nc.vector.tensor_add(s0[:], s0c[:], em_p[:, :, 0])
        c0 = sm_p.tile([BS, 1], F32, tag="c0")
        nc.vector.reduce_max(c0[:], s0[:], axis=AX.X)
        nc0 = sm_p.tile([BS, 1], F32, tag="nc0")
        nc.vector.tensor_scalar_mul(nc0[:], c0[:], -1.0)
        a0 = sm_p.tile([BS, 3], F32, tag="a0")
        nc.scalar.activation(a0[:], s0[:], AF.Exp, bias=nc0[:, 0:1])
        G_r = cur[:, 0, :].rearrange("p (kk jj) -> p jj kk", jj=3)
        aT = sm_p.tile([BS, 3], F32, tag="aT")
        scr3b = sm_p.tile([BS, 3], F32, tag="scr3b")
        for j in range(3):
            nc.vector.tensor_mul(scr3b[:], a0[:], G_r[:, j, :])
            nc.vector.tensor_reduce(aT[:, j:j + 1], scr3b[:], axis=AX.X,
                                    op=OP.add)
        eendc = consts.tile([BS, 3], F32)
        for j in range(3):
            nc.vector.memset(eendc[:, j:j + 1], float(np.exp(end[j])))
        zv = sm_p.tile([BS, 1], F32, tag="zv")
        nc.vector.tensor_mul(scr3b[:], aT[:], eendc[:])
        nc.vector.tensor_reduce(zv[:], scr3b[:], axis=AX.X, op=OP.add)
        lgz = sm_p.tile([BS, 1], F32, tag="lgz")
        nc.scalar.activation(lgz[:], zv[:], AF.Ln)
        denom = sm_p.tile([BS, 1], F32, tag="denom")
        nc.vector.tensor_add(denom[:], lgz[:], ls[:, 0:1])
        nc.vector.tensor_add(denom[:], denom[:], c0[:])

        llh = sm_p.tile([BS, 1], F32, tag="llh")
        nc.vector.tensor_add(llh[:], firstv[:], emtag[:, 0:1])
        nc.vector.tensor_add(llh[:], llh[:], numsum[:])
        nc.vector.tensor_add(llh[:], llh[:], lastv[:])
        nc.vector.tensor_sub(llh[:], llh[:], denom[:])
        nc.sync.dma_start(out_d[:].rearrange("(b o) -> b o", o=1), llh[:])

    return nc


def split_waits(nc, max_waits=1):
    """Walrus in this toolchain accepts only one sync-wait per instruction;
    move extra waits onto same-engine NoOps (engines execute in order)."""
    n = 0
    for f in nc.m.functions:
        for blk in f.blocks:
            new_insts = []
            for inst in blk.instructions:
                si = getattr(inst, "sync_info", None)
                waits = list(si.on_wait) if si is not None and si.on_wait else []
                if len(waits) > max_waits:
                    for w in waits[:-max_waits]:
                        n += 1
                        nop = mybir.InstNoOp(
                            name=f"W-{n}", ins=[], outs=[])
                        nop.engine = inst.engine
                        nop.sync_info = mybir.SyncInfo(on_wait=[w], on_update=[])
                        new_insts.append(nop)
                    si.on_wait = waits[-max_waits:]
                new_insts.append(inst)
            try:
                blk.instructions = new_insts
            except Exception:
                blk.instructions[:] = new_insts
    return n


def kernel(x, tags, lengths, W1, b1, W2, b2, trans, start, end, trace=False):
    x = np.ascontiguousarray(x, np.float32)
    tags = np.ascontiguousarray(tags, np.int32)
    lengths = np.ascontiguousarray(lengths, np.int32)
    nc = build(trans, start, end, b2)
    split_waits(nc)
    in_maps = []
    for i in range(NCORES):
        s = slice(i * BS, (i + 1) * BS)
        in_maps.append({
            "xt": np.ascontiguousarray(x[s].transpose(2, 0, 1)),
            "tags": tags[s], "lengths": lengths[s],
            "W1": np.ascontiguousarray(W1, np.float32),
            "b1": np.ascontiguousarray(b1, np.float32),
            "W2": np.ascontiguousarray(W2, np.float32),
        })
    res = bass_utils.run_bass_kernel_spmd(
        nc, in_maps, core_ids=list(range(NCORES)), trace=trace)
    llh = np.concatenate([res.results[i]["out"] for i in range(NCORES)])
    loss = np.float32(-(llh.astype(np.float64).sum()) / float(llh.size))
    if trace:
        return loss, res
    return loss


# revision 17
# speedup vs baseline: 1.2380x; 1.2380x over previous
"""Trainium2 Bass kernel for CRF loss (MLP emissions + CRF log-likelihood).

Sharding: data-parallel over B=256 sentences -> 32 per core on 8 cores.
Each core computes llh[32] (per-sentence log-likelihood); host sums and
scales (the "all-reduce" of the scalar loss is the trivial host gather).

CRF denominator: forward algorithm rewritten as a product of 3x3
per-step matrices in linear (exp) domain, reduced with a binary tree
(9 levels for T=512) with per-level max-rescaling (scales accumulate in
log domain) for numerical stability.
"""

import sys

sys.path.insert(0, "/opt/trn_rl_repo")

import numpy as np
from contextlib import ExitStack

import concourse.bass as bass
import concourse.mybir as mybir
import concourse.tile as tile
from concourse.masks import make_identity
from concourse import bass_utils

F32 = mybir.dt.float32
BF16 = mybir.dt.bfloat16
I32 = mybir.dt.int32
AF = mybir.ActivationFunctionType
OP = mybir.AluOpType
AX = mybir.AxisListType

BS, T, D, H, K = 32, 512, 512, 256, 3  # per-core shard
NCORES = 8


def build(trans, start, end, b2, mlp_only=False):
    trans = np.asarray(trans, np.float64)
    start = np.asarray(start, np.float64)
    end = np.asarray(end, np.float64)
    b2 = np.asarray(b2, np.float64)

    nc = bass.Bass()
    xt_d = nc.dram_tensor("xt", [D, BS, T], F32, kind="ExternalInput")
    tg_d = nc.dram_tensor("tags", [BS, T], I32, kind="ExternalInput")
    ln_d = nc.dram_tensor("lengths", [BS], I32, kind="ExternalInput")
    w1_d = nc.dram_tensor("W1", [D, H], F32, kind="ExternalInput")
    b1_d = nc.dram_tensor("b1", [H], F32, kind="ExternalInput")
    w2_d = nc.dram_tensor("W2", [H, K], F32, kind="ExternalInput")
    out_d = nc.dram_tensor("out", [BS], F32, kind="ExternalOutput")
    em_dram = nc.dram_tensor("em_scratch", [K, BS, T], BF16, kind="Internal")

    with tile.TileContext(nc) as tc, ExitStack() as ctx:
        consts = ctx.enter_context(tc.tile_pool(name="consts", bufs=1))
        xt_p = ctx.enter_context(tc.tile_pool(name="xt", bufs=3))
        g_p = ctx.enter_context(tc.tile_pool(name="g", bufs=2))
        ps_h = ctx.enter_context(tc.tile_pool(name="ps_h", bufs=4, space="PSUM"))
        ps_e = ctx.enter_context(tc.tile_pool(name="ps_e", bufs=2, space="PSUM"))
        tree_p = ctx.enter_context(tc.tile_pool(name="tree", bufs=2))
        m0_p = ctx.enter_context(tc.tile_pool(name="m0", bufs=1))
        sm_p = ctx.enter_context(tc.tile_pool(name="small", bufs=2))

        # ---- constants / weights ----
        w1f = consts.tile([128, 4, H], F32)
        nc.sync.dma_start(w1f[:], w1_d[:].rearrange("(dc p) h -> p dc h", p=128))
        w1b = consts.tile([128, 4, H], BF16)
        nc.vector.tensor_copy(w1b[:], w1f[:])
        w2f = consts.tile([128, 2, K], F32)
        nc.sync.dma_start(w2f[:], w2_d[:].rearrange("(hc p) k -> p hc k", p=128))
        w2b = consts.tile([128, 2, K], BF16)
        nc.vector.tensor_copy(w2b[:], w2f[:])
        b1sb = consts.tile([128, 2], F32)
        nc.sync.dma_start(b1sb[:], b1_d[:].rearrange("(hc p) -> p hc", p=128))
        pre_b1 = consts.tile([128, 2], F32)
        nc.scalar.copy(pre_b1[:], b1sb[:])

        em_sb = consts.tile([K, BS * T], BF16)

        # ---- MLP: per sentence (512 tokens) ----
        xt_r = xt_d[:].rearrange("(dc p) b t -> p dc b t", p=128)
        for b in range(BS):
            xT = xt_p.tile([128, 4, T], BF16)  # [d128, dc, tok]
            nc.gpsimd.dma_start(xT[:], xt_r[:, :, b, :])
            g = g_p.tile([128, 2, T], BF16)
            for ht in range(2):
                ph = ps_h.tile([128, T], F32)
                for dc in range(4):
                    nc.tensor.matmul(
                        ph[:], lhsT=w1b[:, dc, ht * 128:(ht + 1) * 128],
                        rhs=xT[:, dc, :], start=(dc == 0), stop=(dc == 3))
                nc.scalar.activation(g[:, ht, :], ph[:], AF.Gelu,
                                     bias=b1sb[:, ht:ht + 1])
            pe = ps_e.tile([K, T], F32)
            for ht in range(2):
                nc.tensor.matmul(pe[:], lhsT=w2b[:, ht, :], rhs=g[:, ht, :],
                                 start=(ht == 0), stop=(ht == 1))
            nc.scalar.copy(em_sb[:, b * T:(b + 1) * T], pe[:])

        # bounce em through DRAM to get [b, k, t] layout (partition = sentence)
        nc.sync.dma_start(em_dram[:].rearrange("k b t -> k (b t)"), em_sb[:])
        em_p = consts.tile([BS, K, T], BF16)
        nc.sync.dma_start(em_p[:], em_dram[:].rearrange("k b t -> b k t"))
        pre_em = consts.tile([BS, 1], BF16)
        nc.scalar.copy(pre_em[:], em_p[:, 0, 0:1])

        if mlp_only:
            zz = consts.tile([BS, 1], F32)
            nc.vector.tensor_copy(zz[:], em_p[:, 0, 0:1])
            nc.sync.dma_start(out_d[:].rearrange("(b o) -> b o", o=1), zz[:])
            return nc
        # ---- masks / tags ----
        im_i = consts.tile([BS, T], I32)
        nc.gpsimd.iota(im_i[:], pattern=[[1, T]], base=0, channel_multiplier=0)
        imf = consts.tile([BS, T], F32)
        nc.vector.tensor_copy(imf[:], im_i[:])
        li = consts.tile([BS, 1], I32)
        nc.sync.dma_start(li[:], ln_d[:].rearrange("(b o) -> b o", o=1))
        lf0 = consts.tile([BS, 1], F32)
        nc.vector.tensor_copy(lf0[:], li[:])
        lf = consts.tile([BS, 1], F32)
        nc.vector.tensor_scalar_max(lf[:], lf0[:], 1.0)
        m1 = consts.tile([BS, T], F32)
        nc.vector.tensor_scalar(m1[:], imf[:], lf[:, 0:1], None, OP.is_lt)
        m2 = consts.tile([BS, T], F32)
        nc.vector.tensor_scalar(m2[:], imf[:], 0.0, None, OP.is_gt)
        mp = consts.tile([BS, T], F32)
        nc.vector.tensor_mul(mp[:], m1[:], m2[:])
        omm = consts.tile([BS, T], F32)  # 1 - mp
        nc.vector.tensor_scalar(omm[:], mp[:], -1.0, 1.0, OP.mult, OP.add)

        tg_i = consts.tile([BS, T], I32)
        nc.sync.dma_start(tg_i[:], tg_d[:])
        tgf = consts.tile([BS, T], F32)
        nc.vector.tensor_copy(tgf[:], tg_i[:])

        # ---- CRF denominator on 128 partitions: (sentence, quarter) ----
        em_p128 = consts.tile([128, K, 128], BF16)
        nc.sync.dma_start(em_p128[:],
                          em_dram[:].rearrange("k b (q t) -> (b q) k t", q=4))
        pre_em2 = consts.tile([128, 1], BF16)
        nc.scalar.copy(pre_em2[:], em_p128[:, 0, 0:1])
        im_dram = nc.dram_tensor("im_scratch", [BS, T], F32, kind="Internal")
        nc.sync.dma_start(im_dram[:], imf[:])
        imf128 = consts.tile([128, 128], F32)
        nc.sync.dma_start(imf128[:],
                          im_dram[:].rearrange("b (q t) -> (b q) t", q=4))
        m1_dram = nc.dram_tensor("m1_scratch", [BS, T], F32, kind="Internal")
        nc.sync.dma_start(m1_dram[:], m1[:])
        m1b = consts.tile([128, 128], F32)
        nc.sync.dma_start(m1b[:],
                          m1_dram[:].rearrange("b (q t) -> (b q) t", q=4))
        m2b = consts.tile([128, 128], F32)
        nc.vector.tensor_scalar(m2b[:], imf128[:], 0.0, None, OP.is_gt)
        mpb = consts.tile([128, 128], F32)
        nc.vector.tensor_mul(mpb[:], m1b[:], m2b[:])
        ommb = consts.tile([128, 128], F32)
        nc.vector.tensor_scalar(ommb[:], mpb[:], -1.0, 1.0, OP.mult, OP.add)

        M0 = m0_p.tile([128, 128, 9], F32, tag="lvl0")
        trb = consts.tile([128, 9], F32)
        for i in range(K):
            for j in range(K):
                nc.vector.memset(trb[:, 3 * i + j:3 * i + j + 1],
                                 float(trans[i, j] + b2[j]))
        for i in range(K):
            for j in range(K):
                nc.scalar.activation(M0[:, :, 3 * i + j], em_p128[:, j, :],
                                     AF.Exp, bias=trb[:, 3 * i + j:3 * i + j + 1])
        for c in range(9):
            nc.vector.tensor_mul(M0[:, :, c], M0[:, :, c], mpb[:])
        for c in (0, 4, 8):
            nc.vector.tensor_add(M0[:, :, c], M0[:, :, c], ommb[:])

        def tree(cur, curN, P, ls, ls_pairs, rescale_at):
            while curN > 1:
                N = curN // 2
                nxt = tree_p.tile([P, max(N, 1), 9], F32, tag=f"nxt{P}")
                tmp = tree_p.tile([P, max(N, 1), 3], F32, tag=f"tmp{P}")
                cur_r = cur[:, 0:curN, :].rearrange(
                    "p (q two) e -> p q two e", two=2)
                B_r = cur_r[:, :, 1, :].rearrange(
                    "p q (kk jj) -> p q jj kk", jj=3)
                for i in range(3):
                    for j in range(3):
                        nc.vector.tensor_mul(
                            tmp[:], cur_r[:, :, 0, 3 * i:3 * i + 3],
                            B_r[:, :, j, :])
                        nc.vector.tensor_reduce(
                            nxt[:, :, 3 * i + j], tmp[:], axis=AX.X, op=OP.add)
                if N in rescale_at:
                    mx = sm_p.tile([P, N], F32, tag=f"mx{P}")
                    nc.vector.reduce_max(mx[:], nxt[:], axis=AX.X)
                    rc = sm_p.tile([P, N], F32, tag=f"rc{P}")
                    nc.vector.reciprocal(rc[:], mx[:])
                    for c in range(9):
                        nc.vector.tensor_mul(nxt[:, :, c], nxt[:, :, c], rc[:])
                    lg = sm_p.tile([P, N], F32, tag=f"lg{P}")
                    nc.scalar.activation(lg[:], mx[:], AF.Ln)
                    if ls is None:
                        ls = lg
                        ls_pairs = N
                    else:
                        ls_n = sm_p.tile([P, N], F32, tag=f"lsn{P}")
                        fold = ls_pairs // N
                        ls_r = ls[:, 0:ls_pairs].rearrange(
                            "p (q k) -> p q k", k=fold)
                        nc.vector.tensor_add(ls_n[:], ls_r[:, :, 0],
                                             ls_r[:, :, 1])
                        for kk in range(2, fold):
                            nc.vector.tensor_add(ls_n[:], ls_n[:],
                                                 ls_r[:, :, kk])
                        nc.vector.tensor_add(ls_n[:], ls_n[:], lg[:])
                        ls = ls_n
                        ls_pairs = N
                cur, curN = nxt, N
            return cur, ls

        cur128, ls128 = tree(M0, 128, 128, None, 0, {16, 1})
        fold_dram = nc.dram_tensor("fold_scratch", [128, 10], F32,
                                  kind="Internal")
        nc.sync.dma_start(fold_dram[:, 0:9], cur128[:, 0, :])
        nc.sync.dma_start(fold_dram[:, 9:10], ls128[:])
        G4 = consts.tile([BS, 4, 9], F32)
        nc.sync.dma_start(
            G4[:], fold_dram[:].rearrange("(b q) m -> b q m", q=4)[:, :, 0:9])
        ls4 = consts.tile([BS, 4], F32)
        nc.sync.dma_start(
            ls4[:], fold_dram[:].rearrange("(b q) m -> b q m", q=4)[:, :, 9])
        ls32 = sm_p.tile([BS, 1], F32, tag="ls32")
        nc.vector.tensor_reduce(ls32[:], ls4[:], axis=AX.X, op=OP.add)
        cur, lsf = tree(G4, 4, BS, None, 0, {1})
        ls = sm_p.tile([BS, 1], F32, tag="lsfin")
        nc.vector.tensor_add(ls[:], ls32[:], lsf[:, 0:1])

        # ---- numerator ----
        ind3 = consts.tile([BS, T, 3], F32)
        for j in range(3):
            nc.vector.tensor_scalar(ind3[:, :, j], tgf[:], float(j), None,
                                    OP.is_equal)
        tmpn = consts.tile([BS, T, 3], F32)
        nc.vector.tensor_mul(tmpn[:], em_p[:].rearrange("p j t -> p t j"),
                             ind3[:])
        emtag = consts.tile([BS, T], F32)
        nc.vector.tensor_reduce(emtag[:], tmpn[:], axis=AX.X, op=OP.add)
        if np.any(b2 != 0):
            b2s = consts.tile([BS, T], F32)
            nc.vector.tensor_scalar(b2s[:], ind3[:, :, 0], float(b2[0]), None,
                                    OP.mult)
            for j in (1, 2):
                u = sm_p.tile([BS, T], F32, tag="scr")
                nc.vector.tensor_scalar(u[:], ind3[:, :, j], float(b2[j]), None,
                                        OP.mult)
                nc.vector.tensor_add(b2s[:], b2s[:], u[:])
            nc.vector.tensor_add(emtag[:], emtag[:], b2s[:])

        prevf = consts.tile([BS, T], F32)
        nc.vector.memset(prevf[:, 0:1], 0.0)
        nc.vector.tensor_copy(prevf[:, 1:T], tgf[:, 0:T - 1])
        idxf = consts.tile([BS, T], F32)
        nc.vector.scalar_tensor_tensor(idxf[:], prevf[:], 3.0, tgf[:],
                                       OP.mult, OP.add)
        tr = consts.tile([BS, T], F32)
        tf = trans.reshape(9)
        nc.vector.tensor_scalar(tr[:], idxf[:], 0.0, float(tf[0]),
                                OP.is_equal, OP.mult)
        for p in range(1, 9):
            u2 = sm_p.tile([BS, T], F32, tag="scr")
            nc.vector.tensor_scalar(u2[:], idxf[:], float(p), float(tf[p]),
                                    OP.is_equal, OP.mult)
            nc.vector.tensor_add(tr[:], tr[:], u2[:])
        nc.vector.tensor_add(tr[:], tr[:], emtag[:])
        scrap = consts.tile([BS, T], F32)
        numsum = sm_p.tile([BS, 1], F32, tag="numsum")
        nc.vector.tensor_mul(scrap[:], tr[:], mp[:])
        nc.vector.tensor_reduce(numsum[:], scrap[:], axis=AX.X, op=OP.add)

        startc = consts.tile([BS, 3], F32)
        for i in range(3):
            nc.vector.memset(startc[:, i:i + 1], float(start[i]))
        scr3 = sm_p.tile([BS, 3], F32, tag="scr3")
        firstv = sm_p.tile([BS, 1], F32, tag="firstv")
        nc.vector.tensor_mul(scr3[:], ind3[:, 0, :], startc[:])
        nc.vector.tensor_reduce(firstv[:], scr3[:], axis=AX.X, op=OP.add)

        endv = consts.tile([BS, T], F32)
        nc.vector.tensor_scalar(endv[:], tgf[:], 0.0, float(end[0]),
                                OP.is_equal, OP.mult)
        for j in (1, 2):
            u3 = sm_p.tile([BS, T], F32, tag="scr")
            nc.vector.tensor_scalar(u3[:], tgf[:], float(j), float(end[j]),
                                    OP.is_equal, OP.mult)
            nc.vector.tensor_add(endv[:], endv[:], u3[:])
        indL = consts.tile([BS, T], F32)
        nc.vector.tensor_scalar(indL[:], imf[:], lf[:, 0:1], -1.0,
                                OP.subtract, OP.is_equal)
        lastv = sm_p.tile([BS, 1], F32, tag="lastv")
        nc.vector.tensor_mul(scrap[:], endv[:], indL[:])
        nc.vector.tensor_reduce(lastv[:], scrap[:], axis=AX.X, op=OP.add)

        # ---- final: alpha0 through G, combine ----
        s0c = consts.tile([BS, 3], F32)
        for i in range(3):
            nc.vector.memset(s0c[:, i:i + 1], float(start[i] + b2[i]))
        s0 = sm_p.tile([BS, 3], F32, tag="s0")
        nc.vector.tensor_add(s0[:], s0c[:], em_p[:, :, 0])
        c0 = sm_p.tile([BS, 1], F32, tag="c0")
        nc.vector.reduce_max(c0[:], s0[:], axis=AX.X)
        nc0 = sm_p.tile([BS, 1], F32, tag="nc0")
        nc.vector.tensor_scalar_mul(nc0[:], c0[:], -1.0)
        a0 = sm_p.tile([BS, 3], F32, tag="a0")
        nc.scalar.activation(a0[:], s0[:], AF.Exp, bias=nc0[:, 0:1])
        G_r = cur[:, 0, :].rearrange("p (kk jj) -> p jj kk", jj=3)
        aT = sm_p.tile([BS, 3], F32, tag="aT")
        scr3b = sm_p.tile([BS, 3], F32, tag="scr3b")
        for j in range(3):
            nc.vector.tensor_mul(scr3b[:], a0[:], G_r[:, j, :])
            nc.vector.tensor_reduce(aT[:, j:j + 1], scr3b[:], axis=AX.X,
                                    op=OP.add)
        eendc = consts.tile([BS, 3], F32)
        for j in range(3):
            nc.vector.memset(eendc[:, j:j + 1], float(np.exp(end[j])))
        zv = sm_p.tile([BS, 1], F32, tag="zv")
        nc.vector.tensor_mul(scr3b[:], aT[:], eendc[:])
        nc.vector.tensor_reduce(zv[:], scr3b[:], axis=AX.X, op=OP.add)
        lgz = sm_p.tile([BS, 1], F32, tag="lgz")
        nc.scalar.activation(lgz[:], zv[:], AF.Ln)
        denom = sm_p.tile([BS, 1], F32, tag="denom")
        nc.vector.tensor_add(denom[:], lgz[:], ls[:, 0:1])
        nc.vector.tensor_add(denom[:], denom[:], c0[:])

        llh = sm_p.tile([BS, 1], F32, tag="llh")
        nc.vector.tensor_add(llh[:], firstv[:], emtag[:, 0:1])
        nc.vector.tensor_add(llh[:], llh[:], numsum[:])
        nc.vector.tensor_add(llh[:], llh[:], lastv[:])
        nc.vector.tensor_sub(llh[:], llh[:], denom[:])
        nc.sync.dma_start(out_d[:].rearrange("(b o) -> b o", o=1), llh[:])

    return nc


def split_waits(nc, max_waits=1):
    """Walrus in this toolchain accepts only one sync-wait per instruction;
    move extra waits onto same-engine NoOps (engines execute in order)."""
    n = 0
    for f in nc.m.functions:
        for blk in f.blocks:
            new_insts = []
            for inst in blk.instructions:
                si = getattr(inst, "sync_info", None)
                waits = list(si.on_wait) if si is not None and si.on_wait else []
                if len(waits) > max_waits:
                    for w in waits[:-max_waits]:
                        n += 1
                        nop = mybir.InstNoOp(
                            name=f"W-{n}", ins=[], outs=[])
                        nop.engine = inst.engine
                        nop.sync_info = mybir.SyncInfo(on_wait=[w], on_update=[])
                        new_insts.append(nop)
                    si.on_wait = waits[-max_waits:]
                new_insts.append(inst)
            try:
                blk.instructions = new_insts
            except Exception:
                blk.instructions[:] = new_insts
    return n


def kernel(x, tags, lengths, W1, b1, W2, b2, trans, start, end, trace=False):
    x = np.ascontiguousarray(x, np.float32)
    tags = np.ascontiguousarray(tags, np.int32)
    lengths = np.ascontiguousarray(lengths, np.int32)
    nc = build(trans, start, end, b2)
    split_waits(nc)
    in_maps = []
    for i in range(NCORES):
        s = slice(i * BS, (i + 1) * BS)
        in_maps.append({
            "xt": np.ascontiguousarray(x[s].transpose(2, 0, 1)),
            "tags": tags[s], "lengths": lengths[s],
            "W1": np.ascontiguousarray(W1, np.float32),
            "b1": np.ascontiguousarray(b1, np.float32),
            "W2": np.ascontiguousarray(W2, np.float32),
        })
    res = bass_utils.run_bass_kernel_spmd(
        nc, in_maps, core_ids=list(range(NCORES)), trace=trace)
    llh = np.concatenate([res.results[i]["out"] for i in range(NCORES)])
    loss = np.float32(-(llh.astype(np.float64).sum()) / float(llh.size))
    if trace:
        return loss, res
    return loss
